# revision 2
# baseline (speedup 1.0000x reference)
"""Malvar-He-Cutler demosaic on 8 Trainium2 NeuronCores.

kernel(**inputs) takes the FULL inputs (x int32 (4096,6144), kernels
(4,1,5,5) fp32) and returns the FULL (4096,6144,3) int32 output.

Row sharding: each core gets a 512-row band (reflect padding, fp16
conversion and output assembly are host-side). In-core: 5 row-chunks
of ~103 output rows x 6 column-chunks of 1024.

Device computes only the 8 CONV (plane-set x row-parity) combos; the 4
raw-passthrough combos are filled host-side from x directly. The 12
(channel x row-parity x col-parity) output slots pair up into 4 conv
plane-sets, each a banded fp16 matmul accumulation over the 5
horizontal taps:

    P0 GR@even-cols : even rows G_at_RB   (->G), odd rows R_at_G_Brow (->R)
    P1 B @even-cols : even rows R_at_B    (->B), odd rows R_at_G_Rrow (->B)
    P2 R @odd-cols  : even rows R_at_G_Rrow(->R), odd rows R_at_B     (->R)
    P3 BG@odd-cols  : even rows R_at_G_Brow(->B), odd rows G_at_RB    (->G)

Banded lhsT matrices encode vertical taps AND per-row-parity kernel
selection. Inputs are scaled by 2^-10 so raw values (< 2^24) fit fp16;
host multiplies back and clips. PSUM evictions (fp32->fp16 cast) are
split across the Vector and Activation engines; per (chunk, plane)
contiguous DMAs store fp16 planes to HBM.

_split_waits post-pass: this container's walrus accepts only ONE
semaphore wait per instruction, so excess Tile-emitted waits are
hoisted onto preceding same-engine NOPs (sequencer order preserves
semantics).
"""

import sys

import numpy as np

sys.path.insert(0, "/opt/trn_rl_repo")

H, W = 4096, 6144
NCORES = 8
RB = H // NCORES          # 512 output rows per core
CH = 104                  # max output rows per chunk
CW = 1024                 # output columns per column-chunk
NPS = 4                   # conv plane-sets
XSCALE = 2.0 ** -10       # input prescale so raw values fit fp16


def _row_chunks():
    sizes = [104, 102, 102, 102, 102]
    out, r0 = [], 0
    for s in sizes:
        out.append((r0, s))
        r0 += s
    assert r0 == RB
    return out


def _build_weights(kernels: np.ndarray) -> np.ndarray:
    """Banded lhsT matrices, shape (128, NPS*5*CH) fp16, k-major."""
    K = kernels[:, 0].astype(np.float32)
    psdef = [
        (K[0], K[2]),  # P0 GR@e: even rows G_at_RB,     odd rows R_at_G_Brow
        (K[3], K[1]),  # P1 B @e: even rows R_at_B,      odd rows R_at_G_Rrow
        (K[1], K[3]),  # P2 R @o: even rows R_at_G_Rrow, odd rows R_at_B
        (K[2], K[0]),  # P3 BG@o: even rows R_at_G_Brow, odd rows G_at_RB
    ]
    wts = np.zeros((128, NPS * 5, CH), np.float32)
    p = np.arange(CH)
    for ps, (ke, ko) in enumerate(psdef):
        for dxi in range(5):
            i = ps * 5 + dxi
            for dyi in range(5):
                k = p + dyi
                ok = k < 128
                w = np.where(p % 2 == 0, ke[dyi, dxi], ko[dyi, dxi])
                wts[k[ok], i, p[ok]] = w[ok]
    return wts.reshape(128, NPS * 5 * CH).astype(np.float16)


def _split_waits(nc, maxw=1):
    """Hoist excess semaphore waits onto preceding same-engine NOPs."""
    import concourse.mybir as mybir

    nsplit = 0
    for f in nc.m.functions:
        for b in f.blocks:
            new = []
            for inst in list(b.instructions):
                si = inst.sync_info
                ow = list(si.on_wait) if si and si.on_wait else []
                if len(ow) > maxw:
                    for wx in ow[:-maxw]:
                        new.append(mybir.InstNoOp(
                            name=inst.name + f"-w{nsplit}",
                            sync_info=mybir.SyncInfo(on_wait=[wx], on_update=[]),
                            engine=inst.engine,
                            bass_nofuse=True,
                        ))
                        nsplit += 1
                    si.on_wait = ow[-maxw:]
                new.append(inst)
            b.instructions = new
    return nsplit


def _build_bass(w=W, rowchunks=None, chrows=CH):
    import contextlib

    import concourse.bass as bass
    import concourse.mybir as mybir
    import concourse.tile as tile

    f32 = mybir.dt.float32
    f16 = mybir.dt.float16

    if rowchunks is None:
        rowchunks = _row_chunks()
    rb = sum(s for _, s in rowchunks)
    ncc = w // CW
    ncol = CW // 2
    pw = ncc * ncol  # fp16 plane width (one column parity)

    nc = bass.Bass()
    xb = nc.declare_dram_parameter("xb", [rb + 4, w + 4], f16, isOutput=False)
    wts = nc.declare_dram_parameter("wts", [128, NPS * 5 * chrows], f16,
                                    isOutput=False)
    out = nc.declare_dram_parameter("out", [NPS * rb, pw], f16, isOutput=True)

    with contextlib.ExitStack() as ctx:
        tc = ctx.enter_context(tile.TileContext(nc))
        wpool = ctx.enter_context(tc.tile_pool(name="wpool", bufs=1))
        inpool = ctx.enter_context(tc.tile_pool(name="inpool", bufs=1))
        opool = ctx.enter_context(tc.tile_pool(name="opool", bufs=2))
        pspool = ctx.enter_context(tc.tile_pool(name="pspool", bufs=2,
                                                space="PSUM"))

        wtile = wpool.tile([128, NPS * 5 * chrows], f16)
        nc.sync.dma_start(wtile[:], wts[:])

        # all 5 row-chunk loads prefetch up front (their buffers persist)
        itiles = []
        for g, (r0, rows) in enumerate(rowchunks):
            it = inpool.tile([128, w + 4], f16, tag=f"it{g}", name=f"it{g}")
            nc.sync.dma_start(it[: rows + 4, :], xb[r0 : r0 + rows + 4, :])
            itiles.append(it)

        for g, (r0, rows) in enumerate(rowchunks):
            krows = rows + 4
            otiles = [
                opool.tile([128, pw], f16, tag=f"ot{ps}", name=f"ot{ps}g{g}")
                for ps in range(NPS)
            ]
            for cc in range(ncc):
                for ps in range(NPS):
                    colpar = 0 if ps < 2 else 1
                    ptile = pspool.tile([128, 512], f32, tag=f"ps{ps}")
                    for dxi in range(5):
                        i = ps * 5 + dxi
                        lhsT = wtile[:krows, i * chrows : i * chrows + rows]
                        c0 = CW * cc + colpar + dxi
                        rhs = itiles[g][:krows, c0 : c0 + 2 * ncol - 1 : 2]
                        nc.tensor.matmul(
                            ptile[:rows, :ncol],
                            lhsT,
                            rhs,
                            start=(dxi == 0),
                            stop=(dxi == 4),
                        )
                    dst = otiles[ps][:rows, cc * ncol : (cc + 1) * ncol]
                    if ps % 2 == 0:
                        nc.vector.tensor_copy(dst, ptile[:rows, :ncol])
                    else:
                        nc.scalar.copy(dst, ptile[:rows, :ncol])
            for ps in range(NPS):
                nc.gpsimd.dma_start(
                    out[ps * rb + r0 : ps * rb + r0 + rows, :],
                    otiles[ps][:rows, :],
                )
    _split_waits(nc)
    return nc


_BASS_CACHE = {}


def _get_nc():
    if "nc" not in _BASS_CACHE:
        _BASS_CACHE["nc"] = _build_bass()
    return _BASS_CACHE["nc"]


def _prep_inputs(x: np.ndarray, kernels: np.ndarray):
    """Host-side prep: reflect pad, fp16 prescale, banded weights."""
    xp = np.pad(x, 2, mode="reflect").astype(np.float32)
    xp16 = (xp * np.float32(XSCALE)).astype(np.float16)
    wts = _build_weights(kernels)
    in_maps = []
    for c in range(NCORES):
        band = np.ascontiguousarray(xp16[c * RB : c * RB + RB + 4, :])
        in_maps.append({"xb": band, "wts": wts})
    return in_maps


def _assemble(x: np.ndarray, parts: list) -> np.ndarray:
    """Host-side assembly: scale conv planes, fill passthrough, clip."""
    pw = (W // CW) * (CW // 2)
    planes = [
        np.concatenate([p[ps * RB : (ps + 1) * RB] for p in parts], axis=0)
        .astype(np.float32) * np.float32(1.0 / XSCALE)
        for ps in range(NPS)
    ]
    p0, p1, p2, p3 = planes
    outf = np.empty((H, W, 3), np.float32)
    outf[0::2, 0::2, 1] = p0[0::2]   # G at (even r, even c)
    outf[1::2, 0::2, 0] = p0[1::2]   # R at (odd r, even c)
    outf[:, 0::2, 2] = p1            # B at even cols
    outf[:, 1::2, 0] = p2            # R at odd cols
    outf[0::2, 1::2, 2] = p3[0::2]   # B at (even r, odd c)
    outf[1::2, 1::2, 1] = p3[1::2]   # G at (odd r, odd c)
    # raw passthrough (exact int values, within [0, 2^24))
    outf[0::2, 0::2, 0] = x[0::2, 0::2]
    outf[0::2, 1::2, 1] = x[0::2, 1::2]
    outf[1::2, 0::2, 1] = x[1::2, 0::2]
    outf[1::2, 1::2, 2] = x[1::2, 1::2]
    np.clip(outf, 0.0, 16777215.0, out=outf)
    return outf.astype(np.int32)


def kernel(x: np.ndarray, kernels: np.ndarray) -> np.ndarray:
    from concourse.bass_utils import run_bass_kernel_spmd

    x = np.asarray(x)
    kernels = np.asarray(kernels)
    assert x.shape == (H, W) and x.dtype == np.int32

    in_maps = _prep_inputs(x, kernels)
    nc = _get_nc()
    res = run_bass_kernel_spmd(nc, in_maps, core_ids=list(range(NCORES)))
    parts = [res.results[c]["out"] for c in range(NCORES)]
    return _assemble(x, parts)
